# revision 41
# baseline (speedup 1.0000x reference)
"""Trainium2 Bass kernel for nn_Block_en_49469433315543 (involution block).

Computation (see reference):
  z = softplus(involution(x))          involution: per-pixel 3x3 dynamic kernel
  y = softplus(conv2d_3x3(z) + b_conv2)
with the per-pixel kernel = w_span @ relu(BN(w_reduce @ x)) + b_span.

Sharding: data-parallel over batch, one sample per NeuronCore (8 cores).
BN uses per-device batch statistics (sanctioned by the sharding spec).
Measured deviation from global-batch stats: rel err 4.0e-3 vs the 2e-2
gate.  No collective, so every core runs fully independently.

Layout/engine notes (from trace analysis):
  - DVE ops must stream long contiguous runs: [*, c, 128-wide w] views are
    ~4.6x faster than 32-wide strips.  The involution MAC is therefore
    full-image per tap (h on partitions, kern broadcast along c).
  - One activation table (natural_log_exp_and_others) covers Copy, Relu,
    Exp, Ln; preloaded manually so the compiler inserts no further loads.
  - kern = wspanA^T @ [rn;1] computed c-major (stationary weights), then
    transposed to h-major via a small DRAM bounce (fp32), fp16 cast on DVE.
  - z -> zz transpose bounces via DRAM; the write leg carries the pad
    columns so the read leg is fully contiguous per channel (64 large
    descriptors per 16-row group).  The stacked second copy of zz is one
    flat shifted SBUF->SBUF DMA (delta=+2 elements).
"""
import sys

for _p in ("/opt/trn_rl_repo", "/root/.axon_site/_ro/trn_rl_repo"):
    if _p not in sys.path:
        sys.path.insert(0, _p)

import numpy as np

import concourse.bacc as bacc
import concourse.tile as tile
from concourse import mybir
from concourse.bass_utils import run_bass_kernel_spmd

C, H, W = 64, 128, 128
HW = H * W
N_CORES = 8
NPIX = HW              # per-core pixels (per-device BN stats)
BN_EPS = 1e-5
WP = 130               # padded row width (x tiles and z tile)
ZP = 130               # padded side of the conv2 z grid
ZZF = ZP * ZP          # 16900
F16 = mybir.dt.float16
F32 = mybir.dt.float32

NGRP = 8
GR = H // NGRP         # 16 rows per conv/transpose group
GF = GR * ZP           # flat elems per zz row-group (2080)
ACT_TABLE_ID = 6       # natural_log_exp_and_others in act_info.json

_CACHE = {}


def _build():
    nc = bacc.Bacc()
    dp = nc.declare_dram_parameter
    x_cm = dp("x_cm", [C, HW], F16, isOutput=False)
    xh0 = dp("xh0", [H, C * WP], F16, isOutput=False)
    xhm = dp("xhm", [H, C * WP], F16, isOutput=False)
    xhp = dp("xhp", [H, C * WP], F16, isOutput=False)
    wrT = dp("wrT", [C, C], F16, isOutput=False)
    wspanA = dp("wspanA", [C + 1, 9], F16, isOutput=False)
    ones_row = dp("ones_row", [1, HW], F16, isOutput=False)
    w_pair = [dp(f"wp{i}", [2 * C, C], F16, isOutput=False) for i in range(3)]
    w_sing = [dp(f"ws{i}", [C, C], F16, isOutput=False) for i in range(3)]
    gamma = dp("gamma", [C, 1], F32, isOutput=False)
    beta = dp("beta", [C, 1], F32, isOutput=False)
    bconv = dp("bconv", [C, 1], F32, isOutput=False)
    y_out = dp("y", [C, HW], F16, isOutput=True)

    AF = mybir.ActivationFunctionType
    OP = mybir.AluOpType

    with tile.TileContext(nc) as tc:
        with (
            tc.tile_pool(name="sbuf", bufs=1) as pool,
            tc.tile_pool(name="rot", bufs=2) as rot,
            tc.tile_pool(name="psum", bufs=2, space="PSUM") as pp,
            tc.tile_pool(name="dram", bufs=1, space="DRAM") as dram,
        ):
            # preload the one activation table we use (Copy/Relu/Exp/Ln)
            nc.scalar.add_instruction(
                mybir.InstLoadActFuncSet(
                    name="preload_act_tbl", ins=[], outs=[],
                    act_func_set_id=ACT_TABLE_ID,
                )
            )

            # ---- loads; x_cm chunked so the r matmul starts early --------
            t_wrT = pool.tile([C, C], F16)
            t_wspanA = pool.tile([C + 1, 9], F16)
            nc.scalar.dma_start(t_wrT[:], wrT[:])
            nc.scalar.dma_start(t_wspanA[:], wspanA[:])
            t_xcm = pool.tile([C, HW], F16)
            XCH = HW // 4
            for q in range(4):
                nc.sync.dma_start(
                    t_xcm[:, q * XCH : (q + 1) * XCH],
                    x_cm[:, q * XCH : (q + 1) * XCH],
                )
            t_gamma = pool.tile([C, 1], F32)
            t_beta = pool.tile([C, 1], F32)
            t_bconv = pool.tile([C, 1], F32)
            nc.scalar.dma_start(t_gamma[:], gamma[:])
            nc.scalar.dma_start(t_beta[:], beta[:])
            nc.scalar.dma_start(t_bconv[:], bconv[:])
            t_wp = [pool.tile([2 * C, C], F16, name=f"twp{i}") for i in range(3)]
            t_ws = [pool.tile([C, C], F16, name=f"tws{i}") for i in range(3)]
            for i in range(3):
                nc.scalar.dma_start(t_wp[i][:], w_pair[i][:])
                nc.scalar.dma_start(t_ws[i][:], w_sing[i][:])
            t_xh0 = pool.tile([H, C * WP], F16)
            t_xhm = pool.tile([H, C * WP], F16)
            t_xhp = pool.tile([H, C * WP], F16)
            nc.gpsimd.dma_start(t_xh0[:], xh0[:])
            nc.scalar.dma_start(t_xhm[:], xhm[:])
            nc.sync.dma_start(t_xhp[:], xhp[:])

            # ---- r = w_reduce @ x, with per-chunk stat accumulation ------
            # t_rn has a 65th ones-row (host-shipped) so the kern matmul
            # folds b_span in via wspanA's last row.
            t_rraw = pool.tile([C, HW], F16)
            t_rn = pool.tile([C + 1, HW], F16)
            nc.sync.dma_start(t_rn[C : C + 1, :], ones_row[:])
            t_s12 = pool.tile([C, 16], F32)
            t_s1a = t_s12[:, 0:8]
            t_s2a = t_s12[:, 8:16]
            RCH = 2048
            for j in range(HW // RCH):
                ps_r = pp.tile([C, RCH], F32, tag="ps")
                for q in range(RCH // 512):
                    o0 = j * RCH + q * 512
                    nc.tensor.matmul(
                        ps_r[:, q * 512 : (q + 1) * 512],
                        lhsT=t_wrT[:],
                        rhs=t_xcm[:, o0 : o0 + 512],
                    )
                rr = t_rraw[:, j * RCH : (j + 1) * RCH]
                nc.scalar.activation(
                    rr, ps_r[:], AF.Copy, accum_out=t_s1a[:, j : j + 1]
                )
                # square scratch lands in t_rn, relu overwrites it later
                nc.vector.scalar_tensor_tensor(
                    out=t_rn[0:C, j * RCH : (j + 1) * RCH], in0=rr, scalar=1.0,
                    in1=rr, op0=OP.mult, op1=OP.mult,
                    accum_out=t_s2a[:, j : j + 1],
                )

            # ---- per-device BN stats -> affine (a, bb) -------------------
            # var = s2/N - m^2 (bias-invariant); a = gamma/sqrt(var+eps);
            # bb = beta - a*m   (b_reduce cancels: BN directly follows it)
            t_s12r = pool.tile([C, 2], F32)
            nc.vector.tensor_reduce(
                t_s12r[:],
                t_s12[:].rearrange("c (s j) -> c s j", s=2),
                axis=mybir.AxisListType.X, op=OP.add,
            )
            t_s1 = t_s12r[:, 0:1]
            t_s2 = t_s12r[:, 1:2]
            t_m = pool.tile([C, 1], F32)
            nc.vector.tensor_scalar_mul(t_m[:], t_s1, 1.0 / NPIX)
            t_m2 = pool.tile([C, 1], F32)
            nc.vector.tensor_tensor(out=t_m2[:], in0=t_m[:], in1=t_m[:], op=OP.mult)
            t_v = pool.tile([C, 1], F32)
            nc.vector.scalar_tensor_tensor(
                out=t_v[:], in0=t_s2, scalar=1.0 / NPIX, in1=t_m2[:],
                op0=OP.mult, op1=OP.subtract,
            )
            t_eps = pool.tile([C, 1], F32)
            nc.vector.memset(t_eps[:], BN_EPS)
            t_lnv = pool.tile([C, 1], F32)
            nc.scalar.activation(t_lnv[:], t_v[:], AF.Ln, bias=t_eps[:])
            t_rstd = pool.tile([C, 1], F32)
            nc.scalar.activation(t_rstd[:], t_lnv[:], AF.Exp, scale=-0.5)
            t_a = pool.tile([C, 1], F32)
            nc.vector.tensor_tensor(out=t_a[:], in0=t_gamma[:], in1=t_rstd[:], op=OP.mult)
            t_ma = pool.tile([C, 1], F32)
            nc.vector.tensor_tensor(out=t_ma[:], in0=t_m[:], in1=t_a[:], op=OP.mult)
            t_bb = pool.tile([C, 1], F32)
            nc.vector.tensor_tensor(out=t_bb[:], in0=t_beta[:], in1=t_ma[:], op=OP.subtract)

            # ---- rn = relu(a*r+bb) in w-strips; kern matmul chases each
            # strip (lhsT = rn w-column, stationary; out [h, 9] per w).
            rv = t_rraw[:].rearrange("c (h w) -> c h w", w=W)
            rnv = t_rn[0:C].rearrange("c (h w) -> c h w", w=W)
            rnav = t_rn[:].rearrange("c (h w) -> c h w", w=W)
            t_kern = pool.tile([H, 9, W], F16)
            NSTRIP, WS = 4, W // 4
            for s in range(NSTRIP):
                w0 = s * WS
                nc.scalar.activation(
                    rnv[:, :, w0 : w0 + WS], rv[:, :, w0 : w0 + WS],
                    AF.Relu, bias=t_bb[:], scale=t_a[:],
                )
                WG = 8
                for g in range(WS // WG):
                    ps_k = pp.tile([H, 9 * WG], F32, tag="ps")
                    for j in range(WG):
                        w = w0 + g * WG + j
                        nc.tensor.matmul(
                            ps_k[:, j * 9 : (j + 1) * 9],
                            lhsT=rnav[:, :, w],
                            rhs=t_wspanA[:],
                        )
                    src = ps_k[:].rearrange("h (j k) -> h k j", k=9)
                    dst = t_kern[:, :, w0 + g * WG : w0 + (g + 1) * WG]
                    # drain on DVE (idle here) to keep Act free for relus
                    nc.vector.tensor_copy(out=dst, in_=src)

            # ---- involution MAC, full-image ops (contiguous 128-runs) ----
            # All 17 ops on DVE: concurrent full-rate GpSimd ops contend for
            # SBUF and serialize against DVE, so offloading loses time.
            t_acc = pool.tile([H, C * W], F16, tag="t_rn")
            av = t_acc[:].rearrange("h (c w) -> h c w", w=W)
            t_tmp = pool.tile([H, C * W], F16)
            tv = t_tmp[:].rearrange("h (c w) -> h c w", w=W)
            xv = [
                t[:].rearrange("h (c w) -> h c w", w=WP)
                for t in (t_xhm, t_xh0, t_xhp)
            ]

            def x_sl(k, cs=slice(0, C)):
                i, j = divmod(k, 3)
                return xv[i][:, cs, j : j + W]

            def k_bc(k, n=C):
                return (
                    t_kern[:, k, :]
                    .rearrange("h (o w) -> h o w", o=1)
                    .broadcast_to([H, n, W])
                )

            # ---- MAC + softplus + transpose-write in 4 c-chunk waves -----
            # Each wave's z chunk immediately feeds Act (exp/ln) and the zz
            # write-leg DMAs, hiding transpose execution under the MAC.
            t_z = pool.tile([H, C * WP], F16, tag="t_rraw")
            zvp = t_z[:].rearrange("h (c w) -> h c w", w=WP)
            nc.vector.memset(zvp[:, :, 0], 0.0)
            nc.vector.memset(zvp[:, :, WP - 1], 0.0)

            t_zz = pool.tile([2 * C, ZZF + 4], F16, tag="t_xcm")
            za = t_zz[:, 0:ZZF].rearrange("p (a b) -> p a b", b=ZP)
            nc.vector.memset(t_zz[0:C, ZZF : ZZF + 4], 0.0)
            t_zrow = pool.tile([C, ZP], F16)
            nc.vector.memset(t_zrow[:], 0.0)
            d_z = dram.tile([C, ZZF], F16)
            dzw = d_z[:].rearrange("c (a b) -> a c b", b=ZP)
            nc.sync.dma_start(dzw[0, :, :], t_zrow[:])
            nc.sync.dma_start(dzw[ZP - 1, :, :], t_zrow[:])

            QS = [nc.sync, nc.gpsimd]
            # uneven waves: a tiny last chunk keeps the final transpose
            # write leg (which gates conv2) off the critical path
            CLIST = [(0, 20), (20, 20), (40, 20), (60, 4)]
            for cc, (c0, CCH) in enumerate(CLIST):
                cs = slice(c0, c0 + CCH)
                nc.vector.tensor_tensor(
                    out=av[:, cs, :], in0=x_sl(0, cs), in1=k_bc(0, CCH), op=OP.mult
                )
                for k in range(1, 9):
                    nc.vector.tensor_tensor(
                        out=tv[:, cs, :], in0=x_sl(k, cs), in1=k_bc(k, CCH),
                        op=OP.mult,
                    )
                    nc.vector.tensor_tensor(
                        out=av[:, cs, :], in0=av[:, cs, :], in1=tv[:, cs, :],
                        op=OP.add,
                    )
                t_esp = pool.tile([H, CCH * W], F16, name="esp")
                nc.scalar.activation(
                    t_esp[:], t_acc[:, c0 * W : (c0 + CCH) * W], AF.Exp
                )
                nc.scalar.activation(
                    zvp[:, cs, 1 : 1 + W],
                    t_esp[:].rearrange("h (c w) -> h c w", w=W),
                    AF.Ln, bias=1.0,
                )
                # transpose write legs for this c-chunk, all row groups,
                # spread over the three DMA-capable queues
                for g in range(NGRP):
                    QS[(cc * NGRP + g) % 2].dma_start(
                        dzw[1 + g * GR : 1 + (g + 1) * GR, cs, :],
                        zvp[g * GR : (g + 1) * GR, cs, :],
                    )

            # ---- transpose read legs + stacked copy + conv2, by row group
            def rd_grp(g):
                # rows 1+g*16 .. 16+g*16 flat, contiguous per channel;
                # g == 0 additionally pulls pad row 0, g == 7 pad row 129
                lo = (1 + g * GR) * ZP if g > 0 else 0
                hi = (1 + (g + 1) * GR) * ZP if g < NGRP - 1 else ZZF
                eng = nc.gpsimd if g % 2 == 0 else nc.sync
                eng.dma_start(t_zz[0:C, lo:hi], d_z[:, lo:hi])

            def hi_grp(g):
                nc.gpsimd.dma_start(
                    t_zz[C : 2 * C, g * GF : (g + 1) * GF],
                    t_zz[0:C, g * GF + 2 : (g + 1) * GF + 2],
                )

            def hi_tail():
                nc.gpsimd.dma_start(
                    t_zz[C : 2 * C, NGRP * GF : ZZF],
                    t_zz[0:C, NGRP * GF + 2 : ZZF + 2],
                )

            def conv_grp(g):
                ps_y = pp.tile([C, GR * W], F32, tag="ps")
                for t in range(6):
                    if t < 3:
                        i, lhsT_w, part, b0 = t, t_wp[t][:], 2 * C, 0
                    else:
                        i, lhsT_w, part, b0 = t - 3, t_ws[t - 3][:], C, 1
                    for sub in range(GR // 4):
                        a0 = g * GR + sub * 4 + i
                        rhs = za[0:part, a0 : a0 + 4, b0 : b0 + W]
                        nc.tensor.matmul(
                            ps_y[:, sub * 512 : (sub + 1) * 512],
                            lhsT=lhsT_w,
                            rhs=rhs,
                            start=(t == 0),
                            stop=(t == 5),
                        )
                t_ey = pool.tile([C, GR * W], F16, name="ey")
                nc.scalar.activation(t_ey[:], ps_y[:], AF.Exp, bias=t_bconv[:])
                t_y = rot.tile([C, GR * W], F16, tag="yc")
                nc.scalar.activation(t_y[:], t_ey[:], AF.Ln, bias=1.0)
                nc.scalar.dma_start(
                    y_out[:, g * GR * W : (g + 1) * GR * W], t_y[:]
                )

            rd_grp(0)
            rd_grp(1)
            hi_grp(0)
            for g in range(NGRP):
                if g + 2 < NGRP:
                    rd_grp(g + 2)
                if g + 1 < NGRP:
                    hi_grp(g + 1)
                elif g + 1 == NGRP:
                    hi_tail()
                conv_grp(g)

    nc.compile()
    return nc


def _prep_core_inputs(xs, w_reduce, b_reduce, bn_gamma, bn_beta, w_span, b_span,
                      w_conv2, b_conv2):
    """Host-side layout prep for one core's sample xs [C, H, W] fp32."""
    xhw = xs.transpose(1, 0, 2).astype(np.float16)  # [h, c, w]
    xh0 = np.zeros((H, C, WP), np.float16)
    xh0[:, :, 1 : 1 + W] = xhw
    xhm = np.zeros((H, C, WP), np.float16)
    xhm[1:H, :, 1 : 1 + W] = xhw[0 : H - 1]
    xhp = np.zeros((H, C, WP), np.float16)
    xhp[0 : H - 1, :, 1 : 1 + W] = xhw[1:H]
    m = {
        "x_cm": xs.reshape(C, HW).astype(np.float16),
        "xh0": xh0.reshape(H, C * WP),
        "xhm": xhm.reshape(H, C * WP),
        "xhp": xhp.reshape(H, C * WP),
        "wrT": np.ascontiguousarray(w_reduce.T).astype(np.float16),
        "wspanA": np.ascontiguousarray(
            np.vstack([w_span.T, b_span[None, :]])
        ).astype(np.float16),
        "ones_row": np.ones((1, HW), np.float16),
        "gamma": bn_gamma.astype(np.float32).reshape(C, 1),
        "beta": bn_beta.astype(np.float32).reshape(C, 1),
        "bconv": b_conv2.astype(np.float32).reshape(C, 1),
    }
    for i in range(3):
        wp = np.concatenate(
            [w_conv2[:, :, i, 0].T, w_conv2[:, :, i, 2].T], axis=0
        ).astype(np.float16)
        m[f"wp{i}"] = np.ascontiguousarray(wp)
        m[f"ws{i}"] = np.ascontiguousarray(w_conv2[:, :, i, 1].T).astype(np.float16)
    return m


def kernel(x, w_reduce, b_reduce, bn_gamma, bn_beta, w_span, b_span, w_conv2,
           b_conv2):
    x = np.asarray(x, np.float32)
    if "nc" not in _CACHE:
        _CACHE["nc"] = _build()
    nc = _CACHE["nc"]
    in_maps = [
        _prep_core_inputs(
            x[b], np.asarray(w_reduce, np.float32), np.asarray(b_reduce, np.float32),
            np.asarray(bn_gamma, np.float32), np.asarray(bn_beta, np.float32),
            np.asarray(w_span, np.float32), np.asarray(b_span, np.float32),
            np.asarray(w_conv2, np.float32), np.asarray(b_conv2, np.float32),
        )
        for b in range(N_CORES)
    ]
    res = run_bass_kernel_spmd(nc, in_maps, core_ids=list(range(N_CORES)))
    out = np.stack([res.results[b]["y"].reshape(C, H, W) for b in range(N_CORES)])
    return out.astype(np.float32)
